# revision 24
# baseline (speedup 1.0000x reference)
"""Causal multi-head attention (B=4, T=2048, D=2048, H=16) on 8 Trainium2
NeuronCores via Bass/Tile, SPMD with zero collectives.

Sharding: core = (batch b, head-half hg). Each core owns one batch and 8 of
the 16 heads: it projects Q/K/V for its 1024-column slice of Wq/Wk/Wv over
the full sequence, runs causal attention for its 8 heads, and computes the
partial output projection A @ Wo[hg*1024:(hg+1)*1024, :]. The host feeds
x^T per batch and sums the two partials per batch (bo is folded into the
hg=0 partial on device via a broadcast tile).

All matmul operands are bf16 (cast on the host): bf16 runs the PE at 1
cycle/row like f32r but its LDWEIGHTS uses the fast weight load path (f32
cannot). Accumulation stays f32 in PSUM; softmax denominators accumulate
exactly via per-tile ones-vector matmuls into a dedicated PSUM bank.

The projection and attention phases are FUSED per head: V is projected
first (it is the only projection that needs a DRAM round trip, for the
per-head column gather), then for each head K^T and Q^T are projected
straight into SBUF tiles (no DRAM round trip) and immediately consumed by
that head's attention. Each head's softmax (ACT engine) overlaps the next
head's projection matmuls, so the tensor engine never waits on the scalar
engine. Wo streams in per-head chunks during the head loop.

The host pre-arranges x^T and every weight into the exact SBUF slab layout
([partition, chunk, col]) so every load is a contiguous full-bandwidth
DMA, split across the two hardware queues (SP + Activation).
"""
import numpy as np
import ml_dtypes

import concourse.bacc as bacc
import concourse.mybir as mybir
from concourse.tile import TileContext
from concourse.bass_utils import run_bass_kernel_spmd

F32 = mybir.dt.float32
BF16 = mybir.dt.bfloat16
EXP = mybir.ActivationFunctionType.Exp
MULT = mybir.AluOpType.mult
ADD = mybir.AluOpType.add

PROD_CFG = dict(B=4, T=2048, D=2048, H=16)


def _derived(cfg):
    B, T, D, H = cfg["B"], cfg["T"], cfg["D"], cfg["H"]
    d = dict(cfg)
    d.update(
        HN=H // 2,            # heads per core
        DHD=(H // 2) * (D // H),  # local head dim total (1024)
        DK=D // 128,          # contraction chunks of x^T
        SS=512,               # query supertile width
        DH=D // H,            # 128
        N_CORES=2 * B,
    )
    return d


def build_nc(cfg):
    c = _derived(cfg)
    T, D = c["T"], c["D"]
    HN, DHD, DK, SS = c["HN"], c["DHD"], c["DK"], c["SS"]
    NB = T // 128          # key blocks (16)
    NST = T // SS          # supertiles (4)
    JPS = SS // 128        # key blocks per supertile (4)
    NTQ = T // 512         # t chunks (4)
    SCALE = float(c["DH"] ** -0.5)

    nc = bacc.Bacc(
        "TRN2", target_bir_lowering=False, debug=False, num_devices=c["N_CORES"]
    )
    # host-prearranged slab layouts: [128, chunk*cols] contiguous
    xt = nc.dram_tensor("xt", [128, DK * T], BF16, kind="ExternalInput").ap()
    wkq = nc.dram_tensor(
        "wkq", [128, 2 * HN * DK * 128], BF16, kind="ExternalInput"
    ).ap()
    wv = nc.dram_tensor("wv", [128, DK * DHD], BF16, kind="ExternalInput").ap()
    wo = nc.dram_tensor("wo", [128, HN * D], BF16, kind="ExternalInput").ap()
    bkq = nc.dram_tensor("bkq", [128, 2 * HN], F32, kind="ExternalInput").ap()
    bvb = nc.dram_tensor("bvb", [128, DHD], F32, kind="ExternalInput").ap()
    bob = nc.dram_tensor("bob", [128, D], F32, kind="ExternalInput").ap()
    mask = nc.dram_tensor("mask", [128, 128], BF16, kind="ExternalInput").ap()
    ones_c_in = nc.dram_tensor("ones_c", [128, 1], BF16, kind="ExternalInput").ap()
    o = nc.dram_tensor("o", [T, D], F32, kind="ExternalOutput").ap()

    # per-half V scratch so head-0's reload isn't gated on the full tensor
    v_ds = [
        nc.dram_tensor(f"v_scratch{i}", [T, DHD // 2], BF16).ap()
        for i in range(2)
    ]

    with TileContext(nc) as tc:
        with tc.tile_pool(name="const", bufs=1) as pconst:
            ones_col = pconst.tile([128, 1], BF16, tag="ones_col")
            nc.scalar.dma_start(out=ones_col[:], in_=ones_c_in[:])
            bkq_sb = pconst.tile([128, 2 * HN], F32, tag="bkq")
            nc.scalar.dma_start(out=bkq_sb[:], in_=bkq[:])
            bv_sb = pconst.tile([128, DHD], F32, tag="bv")
            nc.scalar.dma_start(out=bv_sb[:], in_=bvb[:])

            with (
                tc.tile_pool(name="slab", bufs=1) as pslab,
                tc.tile_pool(name="aslab", bufs=1) as paslab,
                tc.tile_pool(name="p3w", bufs=1) as p3w,
                tc.tile_pool(name="p3b", bufs=1) as p3b,
                tc.tile_pool(name="p1w", bufs=2) as p1w,
            ):
                # x^T slab, tq-major [p, tq, k, 512]; the first chunk is
                # split across both DMA queues so compute starts early
                slab = pslab.tile([128, DK * T], BF16, tag="slab")
                slab4 = slab[:].rearrange(
                    "p (tq k t) -> p tq k t", tq=NTQ, k=DK
                )
                xt4 = xt.rearrange("p (tq k t) -> p tq k t", tq=NTQ, k=DK)
                nc.sync.dma_start(
                    out=slab4[:, 0, :DK // 2], in_=xt4[:, 0, :DK // 2]
                )
                nc.scalar.dma_start(
                    out=slab4[:, 0, DK // 2:], in_=xt4[:, 0, DK // 2:]
                )
                for tq in range(1, NTQ):
                    nc.sync.dma_start(out=slab4[:, tq], in_=xt4[:, tq])

                wkq4 = wkq.rearrange(
                    "p (i m r) -> p i m r", i=2, m=HN
                )

                def load_wchunk(pi, m):
                    wc = p1w.tile([128, DK * 128], BF16,
                                  tag=f"wc{pi}")
                    nc.scalar.dma_start(out=wc[:], in_=wkq4[:, pi, m])
                    return wc[:].rearrange("p (k n) -> p k n", k=DK)

                # ---------------- V projection (both halves) ----------------
                with (
                    tc.tile_pool(name="p1wv", bufs=2) as p1wv,
                    tc.tile_pool(name="p1vst", bufs=3) as p1vst,
                    tc.tile_pool(name="ps_v", bufs=2, space="PSUM") as ps_v,
                ):
                    wv3 = wv.rearrange("p (cc r) -> p cc r", cc=2)
                    wvns = []
                    for cc in range(DHD // 512):
                        wvn = p1wv.tile([128, DK * 512], BF16, tag="wvn")
                        wvns.append(wvn[:].rearrange("p (k n) -> p k n", k=DK))
                        nc.scalar.dma_start(out=wvn[:], in_=wv3[:, cc])
                    # K/Q chunks for head 0 arrive during V compute
                    wchunks = [load_wchunk(0, 0), load_wchunk(1, 0)]
                    for cc in range(DHD // 512):
                        wvn3 = wvns[cc]
                        for tb in range(NB):
                            ps = ps_v.tile([128, 512], F32, tag="psv")
                            for k in range(DK):
                                tq, tr = tb // 4, tb % 4
                                nc.tensor.matmul(
                                    ps[:],
                                    slab4[:, tq, k,
                                          tr * 128:(tr + 1) * 128],
                                    wvn3[:, k],
                                    start=(k == 0),
                                    stop=(k == DK - 1),
                                )
                            st = p1vst.tile([128, 512], BF16, tag="vst")
                            nc.vector.tensor_tensor(
                                st[:], ps[:],
                                bv_sb[:, cc * 512:(cc + 1) * 512], ADD,
                            )
                            nc.sync.dma_start(
                                out=v_ds[cc][tb * 128:(tb + 1) * 128, :],
                                in_=st[:],
                            )

                # -------- fused per-head K/Q projection + attention --------
                at_sb = paslab.tile([128, HN * T], BF16, tag="aslab")
                at3 = at_sb[:].rearrange("p (h t) -> p h t", h=HN)
                wo3 = None
                with (
                    tc.tile_pool(name="pmask", bufs=1) as pmask,
                    tc.tile_pool(name="ph", bufs=2) as ph,
                    tc.tile_pool(name="pe", bufs=6) as pe_pool,
                    tc.tile_pool(name="paraw", bufs=4) as paraw,
                    tc.tile_pool(name="psm", bufs=2) as psm,
                    tc.tile_pool(name="plb", bufs=2) as plb,
                    tc.tile_pool(name="ps_kq", bufs=2, space="PSUM") as ps_kq,
                    tc.tile_pool(name="ps_s", bufs=3, space="PSUM") as ps_s,
                    tc.tile_pool(name="ps_a", bufs=2, space="PSUM") as ps_a,
                    tc.tile_pool(name="ps_l", bufs=1, space="PSUM") as ps_l,
                ):
                    mask_sb = pmask.tile([128, 128], BF16, tag="mask")
                    nc.scalar.dma_start(out=mask_sb[:], in_=mask[:])
                    v_drs = [
                        v_d.rearrange("(jb p) c -> p jb c", p=128)
                        for v_d in v_ds
                    ]
                    wo_h = wo.rearrange("p (h n) -> p h n", h=HN)
                    for h in range(HN):
                        # prefetches for this head / the next
                        v_h = ph.tile([128, NB * 128], BF16, tag="vh")
                        v_h3 = v_h[:].rearrange("p (j c) -> p j c", j=NB)
                        hl = (h % 4) * 128
                        nc.sync.dma_start(
                            out=v_h3,
                            in_=v_drs[h // 4][:, :, hl:hl + 128],
                        )
                        if h == 0:
                            bo_sb = p3b.tile([128, D], F32, tag="bo")
                            nc.scalar.dma_start(out=bo_sb[:], in_=bob[:])
                            wo_sb = p3w.tile([128, HN * D], BF16, tag="wo")
                            wo3 = wo_sb[:].rearrange(
                                "p (h n) -> p h n", h=HN
                            )
                        # Wo streams one head-chunk per iteration
                        nc.scalar.dma_start(out=wo3[:, h], in_=wo_h[:, h])

                        # K^T and Q^T straight into SBUF
                        kt_h = ph.tile([128, T], BF16, tag="kth")
                        qt_h = ph.tile([128, T], BF16, tag="qth")
                        wck, wcq = wchunks
                        if h + 1 < HN:
                            wchunks = [load_wchunk(0, h + 1),
                                       load_wchunk(1, h + 1)]
                        for pi, (wc3, dst) in enumerate(
                                ((wck, kt_h), (wcq, qt_h))):
                            for tq in range(NTQ):
                                ps = ps_kq.tile([128, 512], F32, tag="pskq")
                                for k in range(DK):
                                    nc.tensor.matmul(
                                        ps[:],
                                        wc3[:, k],
                                        slab4[:, tq, k],
                                        start=(k == 0),
                                        stop=(k == DK - 1),
                                    )
                                nc.vector.tensor_scalar_add(
                                    dst[:, tq * 512:(tq + 1) * 512], ps[:],
                                    bkq_sb[:, pi * HN + h:pi * HN + h + 1],
                                )

                        def finish_supertile(s, psa, psl):
                            # copy accumulators out fast (frees the PSUM
                            # banks), then the reciprocal/normalize chain
                            araw = paraw.tile([128, SS], F32, tag="araw")
                            nc.vector.tensor_copy(araw[:], psa[:])
                            l_sb = psm.tile([1, SS], F32, tag="lsb")
                            nc.vector.tensor_copy(l_sb[:], psl[:])
                            linv = psm.tile([1, SS], F32, tag="linv")
                            nc.vector.reciprocal_approx_fast(linv[:], l_sb[:])
                            lb = plb.tile([128, SS], F32, tag="lb")
                            nc.gpsimd.partition_broadcast(
                                lb[:], linv[:], channels=128
                            )
                            nc.vector.tensor_tensor(
                                at3[:, h, s * SS:(s + 1) * SS],
                                araw[:], lb[:], MULT,
                            )

                        from collections import deque
                        pending = deque()
                        for s in range(NST):
                            psa = ps_a.tile([128, SS], F32, tag="psa")
                            psl = ps_l.tile([1, SS], F32, tag="psl")
                            nj = JPS * s + JPS
                            for j in range(nj):
                                # diagonal key blocks only score the
                                # queries they can see
                                off = max(0, (j - JPS * s) * 128)
                                w = SS - off
                                pss = ps_s.tile([128, SS], F32, tag="pss")
                                nc.tensor.matmul(
                                    pss[:, :w],
                                    kt_h[:, j * 128:(j + 1) * 128],
                                    qt_h[:, s * SS + off:(s + 1) * SS],
                                    start=True, stop=True,
                                )
                                et = pe_pool.tile([128, SS], BF16, tag="et")
                                nc.scalar.activation(
                                    et[:, :w], pss[:, :w], EXP, scale=SCALE
                                )
                                if j >= JPS * s:
                                    nc.vector.tensor_mul(
                                        et[:, :128], et[:, :128], mask_sb[:]
                                    )
                                if len(pending) >= 3:
                                    pending.popleft()()
                                vj = v_h3[:, j, :]
                                first, last = (j == 0), (j == nj - 1)

                                def consume(et=et, vj=vj, first=first,
                                            last=last, psa=psa, psl=psl,
                                            s=s, off=off, w=w):
                                    nc.tensor.matmul(
                                        psa[:, off:off + w], vj, et[:, :w],
                                        start=first, stop=last,
                                    )
                                    nc.tensor.matmul(
                                        psl[:, off:off + w], ones_col[:],
                                        et[:, :w],
                                        start=first, stop=last,
                                    )
                                    if last:
                                        finish_supertile(s, psa, psl)

                                pending.append(consume)
                        while pending:
                            pending.popleft()()

                # ---------------- output projection ----------------
                with (
                    tc.tile_pool(name="p3st", bufs=3) as p3st,
                    tc.tile_pool(name="ps_o", bufs=2, space="PSUM") as ps_o,
                ):
                    for tb in range(NB):
                        for cc in range(D // 512):
                            pso = ps_o.tile([128, 512], F32, tag="pso")
                            for hh in range(HN):
                                nc.tensor.matmul(
                                    pso[:],
                                    at3[:, hh, tb * 128:(tb + 1) * 128],
                                    wo3[:, hh, cc * 512:(cc + 1) * 512],
                                    start=(hh == 0),
                                    stop=(hh == HN - 1),
                                )
                            ost = p3st.tile([128, 512], F32, tag="ost")
                            nc.vector.tensor_tensor(
                                ost[:], pso[:],
                                bo_sb[:, cc * 512:(cc + 1) * 512], ADD,
                            )
                            nc.sync.dma_start(
                                out=o[tb * 128:(tb + 1) * 128,
                                      cc * 512:(cc + 1) * 512],
                                in_=ost[:],
                            )
    nc.compile()
    return nc


def make_core_inputs(cfg, inputs):
    """Per-core input maps. Core index = 2*b + hg."""
    c = _derived(cfg)
    B, T, D, H = c["B"], c["T"], c["D"], c["H"]
    HN, DHD, DK = c["HN"], c["DHD"], c["DK"]
    f32 = np.float32
    bf16 = ml_dtypes.bfloat16
    x = np.asarray(inputs["x"], f32)
    Wk = np.asarray(inputs["Wk"], f32)
    Wq = np.asarray(inputs["Wq"], f32)
    Wv = np.asarray(inputs["Wv"], f32)
    Wo = np.asarray(inputs["Wo"], f32)
    bk = np.asarray(inputs["bk"], f32)
    bq = np.asarray(inputs["bq"], f32)
    bv = np.asarray(inputs["bv"], f32)
    bo = np.asarray(inputs["bo"], f32)

    p = np.arange(128)[:, None]
    cq = np.arange(128)[None, :]
    mask = (p <= cq).astype(bf16)
    ones_c = np.ones((128, 1), bf16)

    per_hg = []
    for hg in range(2):
        sl = slice(hg * DHD, (hg + 1) * DHD)
        bkq = np.empty((128, 2 * HN), f32)
        bkq[:, :HN] = bk[sl].reshape(HN, 128).T
        bkq[:, HN:] = bq[sl].reshape(HN, 128).T
        # K/Q walls: [p, proj, m, k, n128]
        wall = np.empty((128, 2, HN, DK, 128), f32)
        for pi, W in enumerate((Wk, Wq)):
            ws = W[:, sl].reshape(DK, 128, HN, 128)
            wall[:, pi] = ws.transpose(1, 2, 0, 3)
        # Wo slab: [p, h, n] with p = dh within head h
        wos = Wo[sl, :].reshape(HN, 128, D).transpose(1, 0, 2)
        per_hg.append({
            "wkq": np.ascontiguousarray(wall.reshape(128, -1)).astype(bf16),
            # [p, cc, k, n512] to match the device's (cc, k, n) split
            "wv": np.ascontiguousarray(
                Wv[:, sl].reshape(DK, 128, 2, 512).transpose(1, 2, 0, 3)
                .reshape(128, -1)).astype(bf16),
            "wo": np.ascontiguousarray(wos.reshape(128, -1)).astype(bf16),
            "bkq": bkq,
            "bvb": np.ascontiguousarray(
                np.broadcast_to(bv[sl], (128, DHD))),
            "bob": (np.ascontiguousarray(np.broadcast_to(bo, (128, D)))
                    if hg == 0 else np.zeros((128, D), f32)),
            "mask": mask,
            "ones_c": ones_c,
        })

    in_maps = []
    for b in range(B):
        # x^T slab, tq-major: [p, tq, k, 512]
        xt = x[b].T  # [D, T]
        xts = xt.reshape(DK, 128, T // 512, 512).transpose(1, 2, 0, 3)
        xt_pre = np.ascontiguousarray(xts.reshape(128, -1)).astype(bf16)
        for hg in range(2):
            in_maps.append({"xt": xt_pre, **per_hg[hg]})
    return in_maps


def run_cores(cfg, nc, in_maps, trace=False, tmpdir=None):
    c = _derived(cfg)
    n = c["N_CORES"]
    res = run_bass_kernel_spmd(
        nc, in_maps, list(range(n)), trace=trace, tmpdir=tmpdir
    )
    B, T, D = c["B"], c["T"], c["D"]
    out = np.empty((B, T, D), dtype=np.float32)
    for b in range(B):
        out[b] = res.results[2 * b]["o"]
        out[b] += res.results[2 * b + 1]["o"]
    return out, res


_NC_CACHE = {}


def kernel(x, Wq, bq, Wk, bk, Wv, bv, Wo, bo):
    cfg = PROD_CFG
    key = tuple(sorted(cfg.items()))
    if key not in _NC_CACHE:
        _NC_CACHE[key] = build_nc(cfg)
    nc = _NC_CACHE[key]
    inputs = dict(x=x, Wq=Wq, bq=bq, Wk=Wk, bk=bk, Wv=Wv, bv=bv, Wo=Wo, bo=bo)
    in_maps = make_core_inputs(cfg, inputs)
    out, _ = run_cores(cfg, nc, in_maps)
    return out


# revision 25
# speedup vs baseline: 1.2022x; 1.2022x over previous
"""Causal multi-head attention (B=4, T=2048, D=2048, H=16) on 8 Trainium2
NeuronCores via Bass/Tile, SPMD with zero collectives.

Sharding: core = (batch b, head-half hg). Each core owns one batch and 8 of
the 16 heads: it projects Q/K/V for its 1024-column slice of Wq/Wk/Wv over
the full sequence, runs causal attention for its 8 heads, and computes the
partial output projection A @ Wo[hg*1024:(hg+1)*1024, :]. The host feeds
x^T per batch and sums the two partials per batch (bo is folded into the
hg=0 partial on device via a broadcast tile).

All matmul operands are bf16 (cast on the host): bf16 runs the PE at 1
cycle/row like f32r but its LDWEIGHTS uses the fast weight load path (f32
cannot). Accumulation stays f32 in PSUM; softmax denominators accumulate
exactly via per-tile ones-vector matmuls into a dedicated PSUM bank.

The projection and attention phases are FUSED per head: V is projected
first (it is the only projection that needs a DRAM round trip, for the
per-head column gather), then for each head K^T and Q^T are projected
straight into SBUF tiles (no DRAM round trip) and immediately consumed by
that head's attention. Each head's softmax (ACT engine) overlaps the next
head's projection matmuls, so the tensor engine never waits on the scalar
engine. Wo streams in per-head chunks during the head loop.

The host pre-arranges x^T and every weight into the exact SBUF slab layout
([partition, chunk, col]) so every load is a contiguous full-bandwidth
DMA, split across the two hardware queues (SP + Activation).
"""
import numpy as np
import ml_dtypes

import concourse.bacc as bacc
import concourse.mybir as mybir
from concourse.tile import TileContext
from concourse.bass_utils import run_bass_kernel_spmd

F32 = mybir.dt.float32
BF16 = mybir.dt.bfloat16
EXP = mybir.ActivationFunctionType.Exp
MULT = mybir.AluOpType.mult
ADD = mybir.AluOpType.add

PROD_CFG = dict(B=4, T=2048, D=2048, H=16)


def _derived(cfg):
    B, T, D, H = cfg["B"], cfg["T"], cfg["D"], cfg["H"]
    d = dict(cfg)
    d.update(
        HN=H // 2,            # heads per core
        DHD=(H // 2) * (D // H),  # local head dim total (1024)
        DK=D // 128,          # contraction chunks of x^T
        SS=512,               # query supertile width
        DH=D // H,            # 128
        N_CORES=2 * B,
    )
    return d


def build_nc(cfg):
    c = _derived(cfg)
    T, D = c["T"], c["D"]
    HN, DHD, DK, SS = c["HN"], c["DHD"], c["DK"], c["SS"]
    NB = T // 128          # key blocks (16)
    NST = T // SS          # supertiles (4)
    JPS = SS // 128        # key blocks per supertile (4)
    NTQ = T // 512         # t chunks (4)
    SCALE = float(c["DH"] ** -0.5)

    nc = bacc.Bacc(
        "TRN2", target_bir_lowering=False, debug=False, num_devices=c["N_CORES"]
    )
    # host-prearranged slab layouts: [128, chunk*cols] contiguous
    xt = nc.dram_tensor("xt", [128, DK * T], BF16, kind="ExternalInput").ap()
    wkq = nc.dram_tensor(
        "wkq", [128, 2 * HN * DK * 128], BF16, kind="ExternalInput"
    ).ap()
    wv = nc.dram_tensor("wv", [128, DK * DHD], BF16, kind="ExternalInput").ap()
    wo = nc.dram_tensor("wo", [128, HN * D], BF16, kind="ExternalInput").ap()
    bkq = nc.dram_tensor("bkq", [128, 2 * HN], F32, kind="ExternalInput").ap()
    bvb = nc.dram_tensor("bvb", [128, DHD], F32, kind="ExternalInput").ap()
    bob = nc.dram_tensor("bob", [128, D], F32, kind="ExternalInput").ap()
    mask = nc.dram_tensor("mask", [128, 128], BF16, kind="ExternalInput").ap()
    ones_c_in = nc.dram_tensor("ones_c", [128, 1], BF16, kind="ExternalInput").ap()
    o = nc.dram_tensor("o", [T, D], F32, kind="ExternalOutput").ap()

    # per-half V scratch so head-0's reload isn't gated on the full tensor
    v_ds = [
        nc.dram_tensor(f"v_scratch{i}", [T, DHD // 2], BF16).ap()
        for i in range(2)
    ]

    with TileContext(nc) as tc:
        with tc.tile_pool(name="const", bufs=1) as pconst:
            ones_col = pconst.tile([128, 1], BF16, tag="ones_col")
            nc.scalar.dma_start(out=ones_col[:], in_=ones_c_in[:])
            bkq_sb = pconst.tile([128, 2 * HN], F32, tag="bkq")
            nc.scalar.dma_start(out=bkq_sb[:], in_=bkq[:])
            bv_sb = pconst.tile([128, DHD], F32, tag="bv")

            with (
                tc.tile_pool(name="slab", bufs=1) as pslab,
                tc.tile_pool(name="aslab", bufs=1) as paslab,
                tc.tile_pool(name="p3w", bufs=1) as p3w,
                tc.tile_pool(name="p3b", bufs=1) as p3b,
                tc.tile_pool(name="p1w", bufs=2) as p1w,
            ):
                # x^T slab, tq-major [p, tq, k, 512]; the first chunk is
                # split across both DMA queues so compute starts early
                slab = pslab.tile([128, DK * T], BF16, tag="slab")
                slab4 = slab[:].rearrange(
                    "p (tq k t) -> p tq k t", tq=NTQ, k=DK
                )
                xt4 = xt.rearrange("p (tq k t) -> p tq k t", tq=NTQ, k=DK)
                for tq in range(NTQ):
                    nc.sync.dma_start(out=slab4[:, tq], in_=xt4[:, tq])

                wkq4 = wkq.rearrange(
                    "p (i m r) -> p i m r", i=2, m=HN
                )

                def load_wchunk(pi, m):
                    wc = p1w.tile([128, DK * 128], BF16,
                                  tag=f"wc{pi}")
                    nc.scalar.dma_start(out=wc[:], in_=wkq4[:, pi, m])
                    return wc[:].rearrange("p (k n) -> p k n", k=DK)

                # ---------------- V projection (both halves) ----------------
                with (
                    tc.tile_pool(name="p1wv", bufs=2) as p1wv,
                    tc.tile_pool(name="p1vst", bufs=3) as p1vst,
                    tc.tile_pool(name="ps_v", bufs=2, space="PSUM") as ps_v,
                ):
                    wv3 = wv.rearrange("p (cc r) -> p cc r", cc=2)
                    wvns = []
                    for cc in range(DHD // 512):
                        wvn = p1wv.tile([128, DK * 512], BF16, tag="wvn")
                        wvns.append(wvn[:].rearrange("p (k n) -> p k n", k=DK))
                        nc.scalar.dma_start(out=wvn[:], in_=wv3[:, cc])
                        if cc == 0:
                            nc.scalar.dma_start(out=bv_sb[:], in_=bvb[:])
                    # K/Q chunks for head 0 arrive during V compute
                    wchunks = [load_wchunk(0, 0), load_wchunk(1, 0)]
                    for cc in range(DHD // 512):
                        wvn3 = wvns[cc]
                        for tb in range(NB):
                            ps = ps_v.tile([128, 512], F32, tag="psv")
                            for k in range(DK):
                                tq, tr = tb // 4, tb % 4
                                nc.tensor.matmul(
                                    ps[:],
                                    slab4[:, tq, k,
                                          tr * 128:(tr + 1) * 128],
                                    wvn3[:, k],
                                    start=(k == 0),
                                    stop=(k == DK - 1),
                                )
                            st = p1vst.tile([128, 512], BF16, tag="vst")
                            nc.vector.tensor_tensor(
                                st[:], ps[:],
                                bv_sb[:, cc * 512:(cc + 1) * 512], ADD,
                            )
                            nc.sync.dma_start(
                                out=v_ds[cc][tb * 128:(tb + 1) * 128, :],
                                in_=st[:],
                            )

                # -------- fused per-head K/Q projection + attention --------
                at_sb = paslab.tile([128, HN * T], BF16, tag="aslab")
                at3 = at_sb[:].rearrange("p (h t) -> p h t", h=HN)
                wo3 = None
                with (
                    tc.tile_pool(name="pmask", bufs=1) as pmask,
                    tc.tile_pool(name="ph", bufs=2) as ph,
                    tc.tile_pool(name="pe", bufs=6) as pe_pool,
                    tc.tile_pool(name="paraw", bufs=4) as paraw,
                    tc.tile_pool(name="psm", bufs=2) as psm,
                    tc.tile_pool(name="plb", bufs=2) as plb,
                    tc.tile_pool(name="ps_kq", bufs=2, space="PSUM") as ps_kq,
                    tc.tile_pool(name="ps_s", bufs=3, space="PSUM") as ps_s,
                    tc.tile_pool(name="ps_a", bufs=2, space="PSUM") as ps_a,
                    tc.tile_pool(name="ps_l", bufs=1, space="PSUM") as ps_l,
                ):
                    mask_sb = pmask.tile([128, 128], BF16, tag="mask")
                    nc.scalar.dma_start(out=mask_sb[:], in_=mask[:])
                    v_drs = [
                        v_d.rearrange("(jb p) c -> p jb c", p=128)
                        for v_d in v_ds
                    ]
                    wo_h = wo.rearrange("p (h n) -> p h n", h=HN)
                    for h in range(HN):
                        # prefetches for this head / the next
                        v_h = ph.tile([128, NB * 128], BF16, tag="vh")
                        v_h3 = v_h[:].rearrange("p (j c) -> p j c", j=NB)
                        hl = (h % 4) * 128
                        nc.sync.dma_start(
                            out=v_h3,
                            in_=v_drs[h // 4][:, :, hl:hl + 128],
                        )
                        if h == 0:
                            bo_sb = p3b.tile([128, D], F32, tag="bo")
                            nc.scalar.dma_start(out=bo_sb[:], in_=bob[:])
                            wo_sb = p3w.tile([128, HN * D], BF16, tag="wo")
                            wo3 = wo_sb[:].rearrange(
                                "p (h n) -> p h n", h=HN
                            )
                        # Wo streams one head-chunk per iteration
                        nc.scalar.dma_start(out=wo3[:, h], in_=wo_h[:, h])

                        # K^T and Q^T straight into SBUF
                        kt_h = ph.tile([128, T], BF16, tag="kth")
                        qt_h = ph.tile([128, T], BF16, tag="qth")
                        wck, wcq = wchunks
                        if h + 1 < HN:
                            wchunks = [load_wchunk(0, h + 1),
                                       load_wchunk(1, h + 1)]
                        for pi, (wc3, dst) in enumerate(
                                ((wck, kt_h), (wcq, qt_h))):
                            for tq in range(NTQ):
                                ps = ps_kq.tile([128, 512], F32, tag="pskq")
                                for k in range(DK):
                                    nc.tensor.matmul(
                                        ps[:],
                                        wc3[:, k],
                                        slab4[:, tq, k],
                                        start=(k == 0),
                                        stop=(k == DK - 1),
                                    )
                                nc.vector.tensor_scalar_add(
                                    dst[:, tq * 512:(tq + 1) * 512], ps[:],
                                    bkq_sb[:, pi * HN + h:pi * HN + h + 1],
                                )

                        def finish_supertile(s, psa, psl):
                            # copy accumulators out fast (frees the PSUM
                            # banks), then the reciprocal/normalize chain
                            araw = paraw.tile([128, SS], F32, tag="araw")
                            nc.vector.tensor_copy(araw[:], psa[:])
                            l_sb = psm.tile([1, SS], F32, tag="lsb")
                            nc.vector.tensor_copy(l_sb[:], psl[:])
                            linv = psm.tile([1, SS], F32, tag="linv")
                            nc.vector.reciprocal_approx_fast(linv[:], l_sb[:])
                            lb = plb.tile([128, SS], F32, tag="lb")
                            nc.gpsimd.partition_broadcast(
                                lb[:], linv[:], channels=128
                            )
                            nc.vector.tensor_tensor(
                                at3[:, h, s * SS:(s + 1) * SS],
                                araw[:], lb[:], MULT,
                            )

                        from collections import deque
                        pending = deque()
                        for s in range(NST):
                            psa = ps_a.tile([128, SS], F32, tag="psa")
                            psl = ps_l.tile([1, SS], F32, tag="psl")
                            nj = JPS * s + JPS
                            for j in range(nj):
                                # diagonal key blocks only score the
                                # queries they can see
                                off = max(0, (j - JPS * s) * 128)
                                w = SS - off
                                pss = ps_s.tile([128, SS], F32, tag="pss")
                                nc.tensor.matmul(
                                    pss[:, :w],
                                    kt_h[:, j * 128:(j + 1) * 128],
                                    qt_h[:, s * SS + off:(s + 1) * SS],
                                    start=True, stop=True,
                                )
                                et = pe_pool.tile([128, SS], BF16, tag="et")
                                nc.scalar.activation(
                                    et[:, :w], pss[:, :w], EXP, scale=SCALE
                                )
                                if j >= JPS * s:
                                    nc.vector.tensor_mul(
                                        et[:, :128], et[:, :128], mask_sb[:]
                                    )
                                if len(pending) >= 3:
                                    pending.popleft()()
                                vj = v_h3[:, j, :]
                                first, last = (j == 0), (j == nj - 1)

                                def consume(et=et, vj=vj, first=first,
                                            last=last, psa=psa, psl=psl,
                                            s=s, off=off, w=w):
                                    nc.tensor.matmul(
                                        psa[:, off:off + w], vj, et[:, :w],
                                        start=first, stop=last,
                                    )
                                    nc.tensor.matmul(
                                        psl[:, off:off + w], ones_col[:],
                                        et[:, :w],
                                        start=first, stop=last,
                                    )
                                    if last:
                                        finish_supertile(s, psa, psl)

                                pending.append(consume)
                        while pending:
                            pending.popleft()()

                # ---------------- output projection ----------------
                with (
                    tc.tile_pool(name="p3st", bufs=3) as p3st,
                    tc.tile_pool(name="ps_o", bufs=2, space="PSUM") as ps_o,
                ):
                    for tb in range(NB):
                        for cc in range(D // 512):
                            pso = ps_o.tile([128, 512], F32, tag="pso")
                            for hh in range(HN):
                                nc.tensor.matmul(
                                    pso[:],
                                    at3[:, hh, tb * 128:(tb + 1) * 128],
                                    wo3[:, hh, cc * 512:(cc + 1) * 512],
                                    start=(hh == 0),
                                    stop=(hh == HN - 1),
                                )
                            ost = p3st.tile([128, 512], F32, tag="ost")
                            nc.vector.tensor_tensor(
                                ost[:], pso[:],
                                bo_sb[:, cc * 512:(cc + 1) * 512], ADD,
                            )
                            nc.sync.dma_start(
                                out=o[tb * 128:(tb + 1) * 128,
                                      cc * 512:(cc + 1) * 512],
                                in_=ost[:],
                            )
    nc.compile()
    return nc


def make_core_inputs(cfg, inputs):
    """Per-core input maps. Core index = 2*b + hg."""
    c = _derived(cfg)
    B, T, D, H = c["B"], c["T"], c["D"], c["H"]
    HN, DHD, DK = c["HN"], c["DHD"], c["DK"]
    f32 = np.float32
    bf16 = ml_dtypes.bfloat16
    x = np.asarray(inputs["x"], f32)
    Wk = np.asarray(inputs["Wk"], f32)
    Wq = np.asarray(inputs["Wq"], f32)
    Wv = np.asarray(inputs["Wv"], f32)
    Wo = np.asarray(inputs["Wo"], f32)
    bk = np.asarray(inputs["bk"], f32)
    bq = np.asarray(inputs["bq"], f32)
    bv = np.asarray(inputs["bv"], f32)
    bo = np.asarray(inputs["bo"], f32)

    p = np.arange(128)[:, None]
    cq = np.arange(128)[None, :]
    mask = (p <= cq).astype(bf16)
    ones_c = np.ones((128, 1), bf16)

    per_hg = []
    for hg in range(2):
        sl = slice(hg * DHD, (hg + 1) * DHD)
        bkq = np.empty((128, 2 * HN), f32)
        bkq[:, :HN] = bk[sl].reshape(HN, 128).T
        bkq[:, HN:] = bq[sl].reshape(HN, 128).T
        # K/Q walls: [p, proj, m, k, n128]
        wall = np.empty((128, 2, HN, DK, 128), f32)
        for pi, W in enumerate((Wk, Wq)):
            ws = W[:, sl].reshape(DK, 128, HN, 128)
            wall[:, pi] = ws.transpose(1, 2, 0, 3)
        # Wo slab: [p, h, n] with p = dh within head h
        wos = Wo[sl, :].reshape(HN, 128, D).transpose(1, 0, 2)
        per_hg.append({
            "wkq": np.ascontiguousarray(wall.reshape(128, -1)).astype(bf16),
            # [p, cc, k, n512] to match the device's (cc, k, n) split
            "wv": np.ascontiguousarray(
                Wv[:, sl].reshape(DK, 128, 2, 512).transpose(1, 2, 0, 3)
                .reshape(128, -1)).astype(bf16),
            "wo": np.ascontiguousarray(wos.reshape(128, -1)).astype(bf16),
            "bkq": bkq,
            "bvb": np.ascontiguousarray(
                np.broadcast_to(bv[sl], (128, DHD))),
            "bob": (np.ascontiguousarray(np.broadcast_to(bo, (128, D)))
                    if hg == 0 else np.zeros((128, D), f32)),
            "mask": mask,
            "ones_c": ones_c,
        })

    in_maps = []
    for b in range(B):
        # x^T slab, tq-major: [p, tq, k, 512]
        xt = x[b].T  # [D, T]
        xts = xt.reshape(DK, 128, T // 512, 512).transpose(1, 2, 0, 3)
        xt_pre = np.ascontiguousarray(xts.reshape(128, -1)).astype(bf16)
        for hg in range(2):
            in_maps.append({"xt": xt_pre, **per_hg[hg]})
    return in_maps


def run_cores(cfg, nc, in_maps, trace=False, tmpdir=None):
    c = _derived(cfg)
    n = c["N_CORES"]
    res = run_bass_kernel_spmd(
        nc, in_maps, list(range(n)), trace=trace, tmpdir=tmpdir
    )
    B, T, D = c["B"], c["T"], c["D"]
    out = np.empty((B, T, D), dtype=np.float32)
    for b in range(B):
        out[b] = res.results[2 * b]["o"]
        out[b] += res.results[2 * b + 1]["o"]
    return out, res


_NC_CACHE = {}


def kernel(x, Wq, bq, Wk, bk, Wv, bv, Wo, bo):
    cfg = PROD_CFG
    key = tuple(sorted(cfg.items()))
    if key not in _NC_CACHE:
        _NC_CACHE[key] = build_nc(cfg)
    nc = _NC_CACHE[key]
    inputs = dict(x=x, Wq=Wq, bq=bq, Wk=Wk, bk=bk, Wv=Wv, bv=bv, Wo=Wo, bo=bo)
    in_maps = make_core_inputs(cfg, inputs)
    out, _ = run_cores(cfg, nc, in_maps)
    return out


# revision 27
# speedup vs baseline: 1.2451x; 1.0357x over previous
"""Causal multi-head attention (B=4, T=2048, D=2048, H=16) on 8 Trainium2
NeuronCores via Bass/Tile, SPMD with zero collectives.

Sharding: core = (batch b, head-half hg). Each core owns one batch and 8 of
the 16 heads: it projects Q/K/V for its 1024-column slice of Wq/Wk/Wv over
the full sequence, runs causal attention for its 8 heads, and computes the
partial output projection A @ Wo[hg*1024:(hg+1)*1024, :]. The host feeds
x^T per batch and sums the two partials per batch (bo is folded into the
hg=0 partial on device via a broadcast tile).

All matmul operands are bf16 (cast on the host): bf16 runs the PE at 1
cycle/row like f32r but its LDWEIGHTS uses the fast weight load path (f32
cannot). Accumulation stays f32 in PSUM; softmax denominators accumulate
exactly via per-tile ones-vector matmuls into a dedicated PSUM bank.

The projection and attention phases are FUSED per head: V is projected
first (it is the only projection that needs a DRAM round trip, for the
per-head column gather), then for each head K^T and Q^T are projected
straight into SBUF tiles (no DRAM round trip) and immediately consumed by
that head's attention. Each head's softmax (ACT engine) overlaps the next
head's projection matmuls, so the tensor engine never waits on the scalar
engine. Wo streams in per-head chunks during the head loop.

The host pre-arranges x^T and every weight into the exact SBUF slab layout
([partition, chunk, col]) so every load is a contiguous full-bandwidth
DMA, split across the two hardware queues (SP + Activation).
"""
import numpy as np
import ml_dtypes

import concourse.bacc as bacc
import concourse.mybir as mybir
from concourse.tile import TileContext
from concourse.bass_utils import run_bass_kernel_spmd

F32 = mybir.dt.float32
BF16 = mybir.dt.bfloat16
EXP = mybir.ActivationFunctionType.Exp
MULT = mybir.AluOpType.mult
ADD = mybir.AluOpType.add

PROD_CFG = dict(B=4, T=2048, D=2048, H=16)


def _derived(cfg):
    B, T, D, H = cfg["B"], cfg["T"], cfg["D"], cfg["H"]
    d = dict(cfg)
    d.update(
        HN=H // 2,            # heads per core
        DHD=(H // 2) * (D // H),  # local head dim total (1024)
        DK=D // 128,          # contraction chunks of x^T
        SS=512,               # query supertile width
        DH=D // H,            # 128
        N_CORES=2 * B,
    )
    return d


def build_nc(cfg):
    c = _derived(cfg)
    T, D = c["T"], c["D"]
    HN, DHD, DK, SS = c["HN"], c["DHD"], c["DK"], c["SS"]
    NB = T // 128          # key blocks (16)
    NST = T // SS          # supertiles (4)
    JPS = SS // 128        # key blocks per supertile (4)
    NTQ = T // 512         # t chunks (4)
    SCALE = float(c["DH"] ** -0.5)

    nc = bacc.Bacc(
        "TRN2", target_bir_lowering=False, debug=False, num_devices=c["N_CORES"]
    )
    # host-prearranged slab layouts: [128, chunk*cols] contiguous
    xt = nc.dram_tensor("xt", [128, DK * T], BF16, kind="ExternalInput").ap()
    wkq = nc.dram_tensor(
        "wkq", [128, 2 * HN * DK * 128], BF16, kind="ExternalInput"
    ).ap()
    wv = nc.dram_tensor("wv", [128, DK * DHD], BF16, kind="ExternalInput").ap()
    wo = nc.dram_tensor("wo", [128, HN * D], BF16, kind="ExternalInput").ap()
    bkq = nc.dram_tensor("bkq", [128, 2 * HN], F32, kind="ExternalInput").ap()
    bvb = nc.dram_tensor("bvb", [128, DHD], F32, kind="ExternalInput").ap()
    bob = nc.dram_tensor("bob", [128, D], F32, kind="ExternalInput").ap()
    mask = nc.dram_tensor("mask", [128, 128], BF16, kind="ExternalInput").ap()
    ones_c_in = nc.dram_tensor("ones_c", [128, 1], BF16, kind="ExternalInput").ap()
    o = nc.dram_tensor("o", [T, D], BF16, kind="ExternalOutput").ap()

    # per-half V scratch so head-0's reload isn't gated on the full tensor
    v_ds = [
        nc.dram_tensor(f"v_scratch{i}", [T, DHD // 2], BF16).ap()
        for i in range(2)
    ]

    with TileContext(nc) as tc:
        with tc.tile_pool(name="const", bufs=1) as pconst:
            ones_col = pconst.tile([128, 1], BF16, tag="ones_col")
            nc.scalar.dma_start(out=ones_col[:], in_=ones_c_in[:])
            bkq_sb = pconst.tile([128, 2 * HN], F32, tag="bkq")
            nc.scalar.dma_start(out=bkq_sb[:], in_=bkq[:])
            bv_sb = pconst.tile([128, DHD], F32, tag="bv")

            with (
                tc.tile_pool(name="slab", bufs=1) as pslab,
                tc.tile_pool(name="aslab", bufs=1) as paslab,
                tc.tile_pool(name="p3w", bufs=1) as p3w,
                tc.tile_pool(name="p3b", bufs=1) as p3b,
                tc.tile_pool(name="p1w", bufs=2) as p1w,
            ):
                # x^T slab, tq-major [p, tq, k, 512]; the first chunk is
                # split across both DMA queues so compute starts early
                slab = pslab.tile([128, DK * T], BF16, tag="slab")
                slab4 = slab[:].rearrange(
                    "p (tq k t) -> p tq k t", tq=NTQ, k=DK
                )
                xt4 = xt.rearrange("p (tq k t) -> p tq k t", tq=NTQ, k=DK)
                for tq in range(NTQ):
                    nc.sync.dma_start(out=slab4[:, tq], in_=xt4[:, tq])

                wkq4 = wkq.rearrange(
                    "p (i m r) -> p i m r", i=2, m=HN
                )

                def load_wchunk(pi, m):
                    wc = p1w.tile([128, DK * 128], BF16,
                                  tag=f"wc{pi}")
                    nc.scalar.dma_start(out=wc[:], in_=wkq4[:, pi, m])
                    return wc[:].rearrange("p (k n) -> p k n", k=DK)

                # ---------------- V projection (both halves) ----------------
                with (
                    tc.tile_pool(name="p1wv", bufs=2) as p1wv,
                    tc.tile_pool(name="p1vst", bufs=3) as p1vst,
                    tc.tile_pool(name="ps_v", bufs=2, space="PSUM") as ps_v,
                ):
                    wv3 = wv.rearrange("p (cc r) -> p cc r", cc=2)
                    wvns = []
                    for cc in range(DHD // 512):
                        wvn = p1wv.tile([128, DK * 512], BF16, tag="wvn")
                        wvns.append(wvn[:].rearrange("p (k n) -> p k n", k=DK))
                        nc.scalar.dma_start(out=wvn[:], in_=wv3[:, cc])
                        if cc == 0:
                            nc.scalar.dma_start(out=bv_sb[:], in_=bvb[:])
                    # K/Q chunks for head 0 arrive during V compute
                    wchunks = [load_wchunk(0, 0), load_wchunk(1, 0)]
                    for cc in range(DHD // 512):
                        wvn3 = wvns[cc]
                        for tb in range(NB):
                            ps = ps_v.tile([128, 512], F32, tag="psv")
                            for k in range(DK):
                                tq, tr = tb // 4, tb % 4
                                nc.tensor.matmul(
                                    ps[:],
                                    slab4[:, tq, k,
                                          tr * 128:(tr + 1) * 128],
                                    wvn3[:, k],
                                    start=(k == 0),
                                    stop=(k == DK - 1),
                                )
                            st = p1vst.tile([128, 512], BF16, tag="vst")
                            nc.vector.tensor_tensor(
                                st[:], ps[:],
                                bv_sb[:, cc * 512:(cc + 1) * 512], ADD,
                            )
                            nc.sync.dma_start(
                                out=v_ds[cc][tb * 128:(tb + 1) * 128, :],
                                in_=st[:],
                            )

                # -------- fused per-head K/Q projection + attention --------
                at_sb = paslab.tile([128, HN * T], BF16, tag="aslab")
                at3 = at_sb[:].rearrange("p (h t) -> p h t", h=HN)
                wo3 = None
                with (
                    tc.tile_pool(name="pmask", bufs=1) as pmask,
                    tc.tile_pool(name="ph", bufs=2) as ph,
                    tc.tile_pool(name="pe", bufs=5) as pe_pool,
                    tc.tile_pool(name="paraw", bufs=4) as paraw,
                    tc.tile_pool(name="psm", bufs=2) as psm,
                    tc.tile_pool(name="plb", bufs=2) as plb,
                    tc.tile_pool(name="ps_kq", bufs=2, space="PSUM") as ps_kq,
                    tc.tile_pool(name="ps_s", bufs=3, space="PSUM") as ps_s,
                    tc.tile_pool(name="ps_a", bufs=2, space="PSUM") as ps_a,
                    tc.tile_pool(name="ps_l", bufs=1, space="PSUM") as ps_l,
                ):
                    mask_sb = pmask.tile([128, 128], BF16, tag="mask")
                    nc.scalar.dma_start(out=mask_sb[:], in_=mask[:])
                    v_drs = [
                        v_d.rearrange("(jb p) c -> p jb c", p=128)
                        for v_d in v_ds
                    ]
                    wo_h = wo.rearrange("p (h n) -> p h n", h=HN)
                    for h in range(HN):
                        # prefetches for this head / the next
                        v_h = ph.tile([128, NB * 128], BF16, tag="vh")
                        v_h3 = v_h[:].rearrange("p (j c) -> p j c", j=NB)
                        hl = (h % 4) * 128
                        nc.sync.dma_start(
                            out=v_h3,
                            in_=v_drs[h // 4][:, :, hl:hl + 128],
                        )
                        if h == 0:
                            bo_sb = p3b.tile([128, D], F32, tag="bo")
                            nc.scalar.dma_start(out=bo_sb[:], in_=bob[:])
                            wo_sb = p3w.tile([128, HN * D], BF16, tag="wo")
                            wo3 = wo_sb[:].rearrange(
                                "p (h n) -> p h n", h=HN
                            )
                        # Wo streams one head-chunk per iteration
                        nc.scalar.dma_start(out=wo3[:, h], in_=wo_h[:, h])

                        # K^T and Q^T straight into SBUF
                        kt_h = ph.tile([128, T], BF16, tag="kth")
                        qt_h = ph.tile([128, T], BF16, tag="qth")
                        wck, wcq = wchunks
                        if h + 1 < HN:
                            wchunks = [load_wchunk(0, h + 1),
                                       load_wchunk(1, h + 1)]
                        for pi, (wc3, dst) in enumerate(
                                ((wck, kt_h), (wcq, qt_h))):
                            for tq in range(NTQ):
                                ps = ps_kq.tile([128, 512], F32, tag="pskq")
                                for k in range(DK):
                                    nc.tensor.matmul(
                                        ps[:],
                                        wc3[:, k],
                                        slab4[:, tq, k],
                                        start=(k == 0),
                                        stop=(k == DK - 1),
                                    )
                                nc.vector.tensor_scalar_add(
                                    dst[:, tq * 512:(tq + 1) * 512], ps[:],
                                    bkq_sb[:, pi * HN + h:pi * HN + h + 1],
                                )

                        def finish_supertile(s, psa, psl):
                            # copy accumulators out fast (frees the PSUM
                            # banks), then the reciprocal/normalize chain
                            araw = paraw.tile([128, SS], F32, tag="araw")
                            nc.vector.tensor_copy(araw[:], psa[:])
                            l_sb = psm.tile([1, SS], F32, tag="lsb")
                            nc.vector.tensor_copy(l_sb[:], psl[:])
                            linv = psm.tile([1, SS], F32, tag="linv")
                            nc.vector.reciprocal_approx_fast(linv[:], l_sb[:])
                            lb = plb.tile([128, SS], F32, tag="lb")
                            nc.gpsimd.partition_broadcast(
                                lb[:], linv[:], channels=128
                            )
                            nc.vector.tensor_tensor(
                                at3[:, h, s * SS:(s + 1) * SS],
                                araw[:], lb[:], MULT,
                            )

                        from collections import deque
                        pending = deque()
                        stash = []
                        for s in range(NST):
                            psa = ps_a.tile([128, SS], F32, tag="psa")
                            psl = ps_l.tile([1, SS], F32, tag="psl")
                            nj = JPS * s + JPS
                            for j in range(nj):
                                # diagonal key blocks only score the
                                # queries they can see
                                off = max(0, (j - JPS * s) * 128)
                                w = SS - off
                                pss = ps_s.tile([128, SS], F32, tag="pss")
                                nc.tensor.matmul(
                                    pss[:, :w],
                                    kt_h[:, j * 128:(j + 1) * 128],
                                    qt_h[:, s * SS + off:(s + 1) * SS],
                                    start=True, stop=True,
                                )
                                et = pe_pool.tile([128, SS], BF16, tag="et")
                                nc.scalar.activation(
                                    et[:, :w], pss[:, :w], EXP, scale=SCALE
                                )
                                if j >= JPS * s:
                                    nc.vector.tensor_mul(
                                        et[:, :128], et[:, :128], mask_sb[:]
                                    )
                                if len(pending) >= 3:
                                    pending.popleft()()
                                vj = v_h3[:, j, :]
                                first, last = (j == 0), (j == nj - 1)

                                ndiag = JPS * s  # off-diag count

                                def consume(et=et, vj=vj, first=first,
                                            last=last, psa=psa, psl=psl,
                                            s=s, off=off, w=w, j=j,
                                            ndiag=ndiag):
                                    nc.tensor.matmul(
                                        psa[:, off:off + w], vj, et[:, :w],
                                        start=first, stop=last,
                                    )
                                    if j < ndiag and j % 2 == 0:
                                        # even off-diag: defer, pair with
                                        # the next tile on the DVE
                                        stash.append(et)
                                    else:
                                        if j < ndiag:
                                            prev = stash.pop()
                                            pr = psm.tile(
                                                [128, SS], BF16, tag="pr")
                                            nc.vector.tensor_tensor(
                                                pr[:], prev[:], et[:], ADD)
                                            src_ap = pr[:]
                                        else:
                                            src_ap = et[:, :w]
                                        nc.tensor.matmul(
                                            psl[:, off:off + w], ones_col[:],
                                            src_ap,
                                            start=(j == (1 if ndiag else 0)),
                                            stop=last,
                                        )
                                    if last:
                                        finish_supertile(s, psa, psl)

                                pending.append(consume)
                        while pending:
                            pending.popleft()()

                # ---------------- output projection ----------------
                with (
                    tc.tile_pool(name="p3st", bufs=3) as p3st,
                    tc.tile_pool(name="ps_o", bufs=2, space="PSUM") as ps_o,
                ):
                    for tb in range(NB):
                        for cc in range(D // 512):
                            pso = ps_o.tile([128, 512], F32, tag="pso")
                            for hh in range(HN):
                                nc.tensor.matmul(
                                    pso[:],
                                    at3[:, hh, tb * 128:(tb + 1) * 128],
                                    wo3[:, hh, cc * 512:(cc + 1) * 512],
                                    start=(hh == 0),
                                    stop=(hh == HN - 1),
                                )
                            ost = p3st.tile([128, 512], BF16, tag="ost")
                            nc.vector.tensor_tensor(
                                ost[:], pso[:],
                                bo_sb[:, cc * 512:(cc + 1) * 512], ADD,
                            )
                            nc.sync.dma_start(
                                out=o[tb * 128:(tb + 1) * 128,
                                      cc * 512:(cc + 1) * 512],
                                in_=ost[:],
                            )
    nc.compile()
    return nc


def make_core_inputs(cfg, inputs):
    """Per-core input maps. Core index = 2*b + hg."""
    c = _derived(cfg)
    B, T, D, H = c["B"], c["T"], c["D"], c["H"]
    HN, DHD, DK = c["HN"], c["DHD"], c["DK"]
    f32 = np.float32
    bf16 = ml_dtypes.bfloat16
    x = np.asarray(inputs["x"], f32)
    Wk = np.asarray(inputs["Wk"], f32)
    Wq = np.asarray(inputs["Wq"], f32)
    Wv = np.asarray(inputs["Wv"], f32)
    Wo = np.asarray(inputs["Wo"], f32)
    bk = np.asarray(inputs["bk"], f32)
    bq = np.asarray(inputs["bq"], f32)
    bv = np.asarray(inputs["bv"], f32)
    bo = np.asarray(inputs["bo"], f32)

    p = np.arange(128)[:, None]
    cq = np.arange(128)[None, :]
    mask = (p <= cq).astype(bf16)
    ones_c = np.ones((128, 1), bf16)

    per_hg = []
    for hg in range(2):
        sl = slice(hg * DHD, (hg + 1) * DHD)
        bkq = np.empty((128, 2 * HN), f32)
        bkq[:, :HN] = bk[sl].reshape(HN, 128).T
        bkq[:, HN:] = bq[sl].reshape(HN, 128).T
        # K/Q walls: [p, proj, m, k, n128]
        wall = np.empty((128, 2, HN, DK, 128), f32)
        for pi, W in enumerate((Wk, Wq)):
            ws = W[:, sl].reshape(DK, 128, HN, 128)
            wall[:, pi] = ws.transpose(1, 2, 0, 3)
        # Wo slab: [p, h, n] with p = dh within head h
        wos = Wo[sl, :].reshape(HN, 128, D).transpose(1, 0, 2)
        per_hg.append({
            "wkq": np.ascontiguousarray(wall.reshape(128, -1)).astype(bf16),
            # [p, cc, k, n512] to match the device's (cc, k, n) split
            "wv": np.ascontiguousarray(
                Wv[:, sl].reshape(DK, 128, 2, 512).transpose(1, 2, 0, 3)
                .reshape(128, -1)).astype(bf16),
            "wo": np.ascontiguousarray(wos.reshape(128, -1)).astype(bf16),
            "bkq": bkq,
            "bvb": np.ascontiguousarray(
                np.broadcast_to(bv[sl], (128, DHD))),
            "bob": (np.ascontiguousarray(np.broadcast_to(bo, (128, D)))
                    if hg == 0 else np.zeros((128, D), f32)),
            "mask": mask,
            "ones_c": ones_c,
        })

    in_maps = []
    for b in range(B):
        # x^T slab, tq-major: [p, tq, k, 512]
        xt = x[b].T  # [D, T]
        xts = xt.reshape(DK, 128, T // 512, 512).transpose(1, 2, 0, 3)
        xt_pre = np.ascontiguousarray(xts.reshape(128, -1)).astype(bf16)
        for hg in range(2):
            in_maps.append({"xt": xt_pre, **per_hg[hg]})
    return in_maps


def run_cores(cfg, nc, in_maps, trace=False, tmpdir=None):
    c = _derived(cfg)
    n = c["N_CORES"]
    res = run_bass_kernel_spmd(
        nc, in_maps, list(range(n)), trace=trace, tmpdir=tmpdir
    )
    B, T, D = c["B"], c["T"], c["D"]
    out = np.empty((B, T, D), dtype=np.float32)
    for b in range(B):
        out[b] = np.asarray(res.results[2 * b]["o"], np.float32)
        out[b] += np.asarray(res.results[2 * b + 1]["o"], np.float32)
    return out, res


_NC_CACHE = {}


def kernel(x, Wq, bq, Wk, bk, Wv, bv, Wo, bo):
    cfg = PROD_CFG
    key = tuple(sorted(cfg.items()))
    if key not in _NC_CACHE:
        _NC_CACHE[key] = build_nc(cfg)
    nc = _NC_CACHE[key]
    inputs = dict(x=x, Wq=Wq, bq=bq, Wk=Wk, bk=bk, Wv=Wv, bv=bv, Wo=Wo, bo=bo)
    in_maps = make_core_inputs(cfg, inputs)
    out, _ = run_cores(cfg, nc, in_maps)
    return out
